# revision 11
# baseline (speedup 1.0000x reference)
"""Causal self-attention Trainium2 kernel (8-core SPMD).

Problem: x[2,2048,1024], causal mask, Wqkv[3072,1024], Wo[1024,1024], fp32.
  qkv = x @ Wqkv.T ; per-head causal softmax attention ; out = attn @ Wo.T

Sharding (data + tensor parallel, per the head dimension):
  core c -> batch b = c // 4, heads {4g..4g+3} with g = c % 4.
  Each core computes Q,K,V for its 4 heads (512 qk cols + 256 v cols of the
  projection), runs causal attention for those heads, and multiplies by the
  matching 256 columns of Wo, producing a partial [2048, 1024] output.
  Host sums the 4 partials per batch (the tensor-parallel reduction).

Kernel structure (per core):
  - bf16 matmul operands (PE 1 cyc/row), fp32 PSUM accumulation.
  - Projection chunks (ko-outer, so PE starts as soon as the first 128-row
    slices of x/w arrive) are interleaved with attention chunks: attention
    for q-chunk qc needs only projection chunks nn <= qc, so ACT exp work
    overlaps PE projection matmuls.
  - Scores are computed TRANSPOSED (scoresT[k, q], head pairs packed in the
    PE via partition-base row tiling) so AV needs no transposes. Score
    blocks go into 2-bank PSUM tiles (two k-blocks per tile) so one
    ACTIVATE exps 1024 columns, halving ACT instruction overhead.
  - Causality: strictly-upper blocks skipped; diagonal straddlers compute
    only the valid columns; the 128x128 diagonal sub-block is exp'd
    unmasked then multiplied by a binary mask tile (from the mask input).
  - V carries a ones column (65 cols/head): AV's partition 64 accumulates
    the softmax denominator for free. Normalization = fast-approx
    reciprocal (sums >= 1), broadcast over partitions via a K=1
    ones-matmul, one DVE multiply.
"""

import os

import numpy as np

S = 2048
D = 1024
DH = 64
B = 2
NCORES = 8
HPC = 4  # heads per core
QKC = 2 * HPC * DH  # 512 q+k projection columns per core
VC = HPC * DH  # 256 v columns per core
P = 128
KO = D // P  # 8 contraction tiles
NQ = S // 512  # 4 q-chunks of 512
NSC = S // P  # 16 s-chunks of 128

COMPUTE_DT = os.environ.get("ATTN_COMPUTE_DT", "bf16")  # bf16 | f32r

_cache = {}


def _np_compute_dt():
    if COMPUTE_DT == "bf16":
        import ml_dtypes

        return ml_dtypes.bfloat16
    return np.float32


def _build():
    import concourse.bacc as bacc
    import concourse.mybir as mybir
    import concourse.tile as tile

    F32 = mybir.dt.float32
    CDT = mybir.dt.bfloat16 if COMPUTE_DT == "bf16" else mybir.dt.float32r
    EXP = mybir.ActivationFunctionType.Exp

    nc = bacc.Bacc()
    xT_d = nc.dram_tensor("xT", [D, S], CDT, kind="ExternalInput")
    wqkT_d = nc.dram_tensor("wqkT", [D, QKC], CDT, kind="ExternalInput")
    wvT_d = nc.dram_tensor("wvT", [D, VC], CDT, kind="ExternalInput")
    woT_d = nc.dram_tensor("woT", [VC, D], CDT, kind="ExternalInput")
    maskT_d = nc.dram_tensor("maskT", [P, P], CDT, kind="ExternalInput")
    out_d = nc.dram_tensor("out", [S, D], F32, kind="ExternalOutput")

    with tile.TileContext(nc) as tc:
        with (
            tc.tile_pool(name="persist", bufs=1) as persist,
            tc.tile_pool(name="sb_small", bufs=3) as sb_small,
            tc.tile_pool(name="sb_exp", bufs=10) as sb_exp,
            tc.tile_pool(name="sb_out", bufs=3) as sb_out,
            tc.tile_pool(name="pp_big", bufs=3, space="PSUM") as pp_big,
            tc.tile_pool(name="pp_av", bufs=1, space="PSUM") as pp_av,
            tc.tile_pool(name="pp_o", bufs=1, space="PSUM") as pp_o,
        ):
            xT_sb = persist.tile([P, KO, S], CDT, tag="xT")
            wqkT_sb = persist.tile([P, KO, QKC], CDT, tag="wqkT")
            wvT_sb = persist.tile([P, KO, VC], CDT, tag="wvT")
            woT_sb = persist.tile([P, 2, D], CDT, tag="woT")
            maskT_sb = persist.tile([P, P], CDT, tag="maskT")
            ones_sb = persist.tile([1, DH], CDT, tag="ones")
            qkT_sb = persist.tile([P, 4, S], CDT, tag="qkT")
            v_sb = persist.tile([P, NSC, HPC, DH + 1], CDT, tag="v")
            attn_sb = persist.tile([P, 2, S], CDT, tag="attn")

            # --- input DMAs: wqkT[ko]+xT[ko] pairs first (unblock proj ko
            # chains asap), across two queues; then wvT, mask, woT ---
            for ko in range(KO):
                e1, e2 = (nc.sync, nc.gpsimd) if ko % 2 == 0 else (nc.gpsimd, nc.sync)
                e1.dma_start(wqkT_sb[:, ko, :], wqkT_d[ko * P : (ko + 1) * P, :])
                e2.dma_start(xT_sb[:, ko, :], xT_d[ko * P : (ko + 1) * P, :])
            for ko in range(KO):
                eng = nc.sync if ko % 2 == 0 else nc.gpsimd
                eng.dma_start(wvT_sb[:, ko, :], wvT_d[ko * P : (ko + 1) * P, :])
            nc.sync.dma_start(maskT_sb[:], maskT_d[:])
            nc.gpsimd.dma_start(woT_sb[:], woT_d.rearrange("(ct p) e -> p ct e", p=P))
            ones_f32 = persist.tile([P, DH], F32, tag="ones_f32")
            nc.vector.memset(ones_f32[:], 1.0)
            nc.vector.tensor_copy(out=ones_sb[:], in_=ones_f32[0:1, :])
            nc.vector.tensor_copy(
                out=v_sb[:, :, :, DH],
                in_=ones_f32[:, 0 : NSC * HPC].rearrange("p (a b) -> p a b", a=NSC),
            )

            def emit_outproj(qc):
                for si in range(4):
                    sc = qc * 4 + si
                    for en in range(2):
                        ps_o = pp_o.tile([P, 512], F32, tag="o")
                        for ct in range(2):
                            nc.tensor.matmul(
                                ps_o[:],
                                attn_sb[:, ct, sc * P : (sc + 1) * P],
                                woT_sb[:, ct, en * 512 : (en + 1) * 512],
                                start=(ct == 0),
                                stop=(ct == 1),
                            )
                        o_sb = sb_out.tile([P, 512], F32, tag="osb")
                        nc.vector.tensor_copy(out=o_sb[:], in_=ps_o[:])
                        nc.sync.dma_start(
                            out_d[sc * P : (sc + 1) * P, en * 512 : (en + 1) * 512],
                            o_sb[:],
                        )

            for qc in range(NQ):
                # --- qk projection chunk nn = qc, ko-outer over two 2-bank
                # tiles (4 half-bank chains), so PE tracks DMA arrival ---
                pjA = pp_big.tile([P, 1024], F32, tag="big", name="pjA")
                pjB = pp_big.tile([P, 1024], F32, tag="big", name="pjB")
                for ko in range(KO):
                    for mm in range(4):
                        slot = pjA if mm < 2 else pjB
                        nc.tensor.matmul(
                            slot[:, (mm % 2) * 512 : (mm % 2 + 1) * 512],
                            wqkT_sb[:, ko, mm * P : (mm + 1) * P],
                            xT_sb[:, ko, qc * 512 : (qc + 1) * 512],
                            start=(ko == 0),
                            stop=(ko == KO - 1),
                            skip_group_check=True,
                        )
                nc.vector.tensor_copy(
                    out=qkT_sb[:, 0:2, qc * 512 : (qc + 1) * 512],
                    in_=pjA.rearrange("p (a b) -> p a b", a=2),
                )
                nc.vector.tensor_copy(
                    out=qkT_sb[:, 2:4, qc * 512 : (qc + 1) * 512],
                    in_=pjB.rearrange("p (a b) -> p a b", a=2),
                )

                # --- v projection for s-chunks 4qc..4qc+3 (4 bank chains) ---
                pvA = pp_big.tile([P, 1024], F32, tag="big", name="pvA")
                pvB = pp_big.tile([P, 1024], F32, tag="big", name="pvB")
                for ko in range(KO):
                    for j in range(4):
                        slot = pvA if j < 2 else pvB
                        sc = 4 * qc + j
                        nc.tensor.matmul(
                            slot[:, (j % 2) * 512 : (j % 2) * 512 + VC],
                            xT_sb[:, ko, sc * P : (sc + 1) * P],
                            wvT_sb[:, ko, :],
                            start=(ko == 0),
                            stop=(ko == KO - 1),
                            skip_group_check=True,
                        )
                for half, slot in ((0, pvA), (1, pvB)):
                    nc.vector.tensor_copy(
                        out=v_sb[:, 4 * qc + 2 * half : 4 * qc + 2 * half + 2, :, 0:DH],
                        in_=slot.rearrange("p (a h d) -> p a h d", a=2, h=8)[:, :, 0:HPC, :],
                    )

                # --- attention for q-chunk qc ---
                nkb = 4 * qc + 4  # causal: k blocks 0 .. 4qc+3
                for h in range(HPC):
                    hp = (h % 2) * DH  # partition base within the m-tile
                    mq = h // 2  # Q m-tile; K m-tile = 2 + h//2
                    avs = []
                    for kb0 in range(0, nkb, 2):
                        ps2 = pp_big.tile([P, 1024], F32, tag="big", name="ps2")
                        exp2 = sb_exp.tile([P, 1024], CDT, tag="exp")
                        offs = []
                        for half in (0, 1):
                            kb = kb0 + half
                            m = kb - 4 * qc  # >= 0 on diagonal straddlers
                            off = max(0, m) * P
                            offs.append(off)
                            nc.tensor.matmul(
                                ps2[:, half * 512 + off : half * 512 + 512],
                                qkT_sb[hp : hp + DH, 2 + mq, kb * P : (kb + 1) * P],
                                qkT_sb[hp : hp + DH, mq, qc * 512 + off : (qc + 1) * 512],
                                start=True,
                                stop=True,
                                skip_group_check=True,
                            )
                        if offs[0] == 0 and offs[1] == 0:
                            # dense pair: one 1024-wide exp
                            nc.scalar.activation(exp2[:], ps2[:], EXP, scale=0.125)
                        else:
                            for half, off in enumerate(offs):
                                lo = half * 512 + off
                                nc.scalar.activation(
                                    exp2[:, lo : half * 512 + 512],
                                    ps2[:, lo : half * 512 + 512],
                                    EXP,
                                    scale=0.125,
                                )
                        for half, off in enumerate(offs):
                            kb = kb0 + half
                            if kb - 4 * qc >= 0:
                                lo = half * 512 + off
                                nc.vector.tensor_mul(
                                    out=exp2[:, lo : lo + P],
                                    in0=exp2[:, lo : lo + P],
                                    in1=maskT_sb[:],
                                )
                            avs.append((exp2, half * 512 + off, off, kb))
                    ps_av = pp_av.tile([DH + 1, 512], F32, tag="av")
                    for j, (exp2, lo, off, kb) in enumerate(avs):
                        nc.tensor.matmul(
                            ps_av[:, off:512],
                            v_sb[:, kb, h, :],
                            exp2[:, lo : (lo - off) + 512],
                            start=(j == 0),
                            stop=(j == len(avs) - 1),
                            skip_group_check=True,
                        )
                    # normalize: out = av * (1/sums) broadcast over partitions
                    sums_sb = sb_small.tile([1, 512], F32, tag="sums")
                    nc.vector.tensor_copy(out=sums_sb[:], in_=ps_av[DH : DH + 1, :])
                    recip_f = sb_small.tile([1, 512], F32, tag="recipf")
                    nc.vector.reciprocal_approx_fast(out=recip_f[:], in_=sums_sb[:])
                    recip = sb_small.tile([1, 512], CDT, tag="recip")
                    nc.vector.tensor_copy(out=recip[:], in_=recip_f[:])
                    ps_bt = pp_big.tile([P, 1024], F32, tag="big", name="ps_bt")
                    ps_b = ps_bt[0:DH, 0:512]
                    nc.tensor.matmul(ps_b, ones_sb[:], recip[:], start=True, stop=True)
                    bc_sb = sb_small.tile([DH, 512], F32, tag="bc")
                    nc.vector.tensor_copy(out=bc_sb[:], in_=ps_b)
                    nc.vector.tensor_mul(
                        out=attn_sb[hp : hp + DH, h // 2, qc * 512 : (qc + 1) * 512],
                        in0=ps_av[0:DH, :],
                        in1=bc_sb[:],
                    )

                # --- deferred output projection (previous q chunk) ---
                if qc > 0:
                    emit_outproj(qc - 1)
            emit_outproj(NQ - 1)

    nc.compile()
    return nc


def _get_nc():
    if "nc" not in _cache:
        _cache["nc"] = _build()
    return _cache["nc"]


def _shard(x, mask, Wqkv, Wo):
    cdt = _np_compute_dt()
    in_maps = []
    # binary mask for the transposed 128x128 diagonal block:
    # valid (mask==0) -> 1.0, masked (-inf/large-negative) -> 0.0
    maskT = np.ascontiguousarray((mask[0, 0, :P, :P].T >= 0).astype(cdt))
    for c in range(NCORES):
        b = c // 4
        g = c % 4
        heads = [4 * g + i for i in range(HPC)]
        q_rows = np.concatenate([np.arange(h * DH, (h + 1) * DH) for h in heads])
        k_rows = D + q_rows
        v_rows = 2 * D + q_rows
        qk_rows = np.concatenate([q_rows, k_rows])
        in_maps.append(
            {
                "xT": np.ascontiguousarray(x[b].T.astype(cdt)),
                "wqkT": np.ascontiguousarray(Wqkv[qk_rows, :].T.astype(cdt)),
                "wvT": np.ascontiguousarray(Wqkv[v_rows, :].T.astype(cdt)),
                "woT": np.ascontiguousarray(Wo[:, q_rows].T.astype(cdt)),
                "maskT": maskT,
            }
        )
    return in_maps


def kernel(x, mask, Wqkv, Wo, _trace=False):
    from concourse.bass_utils import run_bass_kernel_spmd

    x = np.asarray(x, dtype=np.float32)
    mask = np.asarray(mask, dtype=np.float32)
    Wqkv = np.asarray(Wqkv, dtype=np.float32)
    Wo = np.asarray(Wo, dtype=np.float32)

    nc = _get_nc()
    in_maps = _shard(x, mask, Wqkv, Wo)
    res = run_bass_kernel_spmd(nc, in_maps, core_ids=list(range(NCORES)), trace=_trace)
    _cache["last_result"] = res

    out = np.zeros((B, S, D), dtype=np.float32)
    for c in range(NCORES):
        out[c // 4] += res.results[c]["out"]
    return out


# revision 12
# speedup vs baseline: 1.2817x; 1.2817x over previous
"""Causal self-attention Trainium2 kernel (8-core SPMD).

Problem: x[2,2048,1024], causal mask, Wqkv[3072,1024], Wo[1024,1024], fp32.
  qkv = x @ Wqkv.T ; per-head causal softmax attention ; out = attn @ Wo.T

Sharding (data + tensor parallel, per the head dimension):
  core c -> batch b = c // 4, heads {4g..4g+3} with g = c % 4.
  Each core computes Q,K,V for its 4 heads (512 qk cols + 256 v cols of the
  projection), runs causal attention for those heads, and multiplies by the
  matching 256 columns of Wo, producing a partial [2048, 1024] output.
  Host sums the 4 partials per batch (the tensor-parallel reduction).

Kernel structure (per core):
  - bf16 matmul operands (PE 1 cyc/row), fp32 PSUM accumulation.
  - Projection chunks (ko-outer, so PE starts as soon as the first 128-row
    slices of x/w arrive) are interleaved with attention chunks: attention
    for q-chunk qc needs only projection chunks nn <= qc, so ACT exp work
    overlaps PE projection matmuls.
  - Scores are computed TRANSPOSED (scoresT[k, q], head pairs packed in the
    PE via partition-base row tiling) so AV needs no transposes. Score
    blocks go into 2-bank PSUM tiles (two k-blocks per tile) so one
    ACTIVATE exps 1024 columns, halving ACT instruction overhead.
  - Causality: strictly-upper blocks skipped; diagonal straddlers compute
    only the valid columns; the 128x128 diagonal sub-block is exp'd
    unmasked then multiplied by a binary mask tile (from the mask input).
  - V carries a ones column (65 cols/head): AV's partition 64 accumulates
    the softmax denominator for free. Normalization = fast-approx
    reciprocal (sums >= 1), broadcast over partitions via a K=1
    ones-matmul, one DVE multiply.
"""

import os

import numpy as np

S = 2048
D = 1024
DH = 64
B = 2
NCORES = 8
HPC = 4  # heads per core
QKC = 2 * HPC * DH  # 512 q+k projection columns per core
VC = HPC * DH  # 256 v columns per core
P = 128
KO = D // P  # 8 contraction tiles
NQ = S // 512  # 4 q-chunks of 512
NSC = S // P  # 16 s-chunks of 128

COMPUTE_DT = os.environ.get("ATTN_COMPUTE_DT", "bf16")  # bf16 | f32r

_cache = {}


def _np_compute_dt():
    if COMPUTE_DT == "bf16":
        import ml_dtypes

        return ml_dtypes.bfloat16
    return np.float32


def _build():
    import concourse.bacc as bacc
    import concourse.mybir as mybir
    import concourse.tile as tile

    F32 = mybir.dt.float32
    CDT = mybir.dt.bfloat16 if COMPUTE_DT == "bf16" else mybir.dt.float32r
    EXP = mybir.ActivationFunctionType.Exp

    nc = bacc.Bacc()
    xT_d = nc.dram_tensor("xT", [D, S], CDT, kind="ExternalInput")
    wqkT_d = nc.dram_tensor("wqkT", [D, QKC], CDT, kind="ExternalInput")
    wvT_d = nc.dram_tensor("wvT", [D, VC], CDT, kind="ExternalInput")
    woT_d = nc.dram_tensor("woT", [VC, D], CDT, kind="ExternalInput")
    maskT_d = nc.dram_tensor("maskT", [P, P], CDT, kind="ExternalInput")
    out_d = nc.dram_tensor("out", [S, D], F32, kind="ExternalOutput")

    with tile.TileContext(nc) as tc:
        with (
            tc.tile_pool(name="persist", bufs=1) as persist,
            tc.tile_pool(name="sb_small", bufs=3) as sb_small,
            tc.tile_pool(name="sb_exp", bufs=10) as sb_exp,
            tc.tile_pool(name="sb_out", bufs=3) as sb_out,
            tc.tile_pool(name="pp_big", bufs=2, space="PSUM") as pp_big,
            tc.tile_pool(name="pp_av", bufs=1, space="PSUM") as pp_av,
            tc.tile_pool(name="pp_b", bufs=1, space="PSUM") as pp_b,
            tc.tile_pool(name="pp_o", bufs=2, space="PSUM") as pp_o,
        ):
            xT_sb = persist.tile([P, KO, S], CDT, tag="xT")
            wqkT_sb = persist.tile([P, KO, QKC], CDT, tag="wqkT")
            wvT_sb = persist.tile([P, KO, VC], CDT, tag="wvT")
            woT_sb = persist.tile([P, 2, D], CDT, tag="woT")
            maskT_sb = persist.tile([P, P], CDT, tag="maskT")
            ones_sb = persist.tile([1, DH], CDT, tag="ones")
            qkT_sb = persist.tile([P, 4, S], CDT, tag="qkT")
            v_sb = persist.tile([P, NSC, HPC, DH + 1], CDT, tag="v")
            attn_sb = persist.tile([P, 2, S], CDT, tag="attn")

            # --- input DMAs: wqkT[ko]+xT[ko] pairs first (unblock proj ko
            # chains asap), across two queues; then wvT, mask, woT ---
            for ko in range(KO):
                e1, e2 = (nc.sync, nc.gpsimd) if ko % 2 == 0 else (nc.gpsimd, nc.sync)
                e1.dma_start(wqkT_sb[:, ko, :], wqkT_d[ko * P : (ko + 1) * P, :])
                e2.dma_start(xT_sb[:, ko, :], xT_d[ko * P : (ko + 1) * P, :])
            for ko in range(KO):
                eng = nc.sync if ko % 2 == 0 else nc.gpsimd
                eng.dma_start(wvT_sb[:, ko, :], wvT_d[ko * P : (ko + 1) * P, :])
            nc.sync.dma_start(maskT_sb[:], maskT_d[:])
            nc.gpsimd.dma_start(woT_sb[:], woT_d.rearrange("(ct p) e -> p ct e", p=P))
            ones_f32 = persist.tile([P, DH], F32, tag="ones_f32")
            nc.vector.memset(ones_f32[:], 1.0)
            nc.vector.tensor_copy(out=ones_sb[:], in_=ones_f32[0:1, :])
            nc.vector.tensor_copy(
                out=v_sb[:, :, :, DH],
                in_=ones_f32[:, 0 : NSC * HPC].rearrange("p (a b) -> p a b", a=NSC),
            )

            def emit_outproj(qc):
                for si in range(4):
                    sc = qc * 4 + si
                    for en in range(2):
                        ps_o = pp_o.tile([P, 512], F32, tag="o")
                        for ct in range(2):
                            nc.tensor.matmul(
                                ps_o[:],
                                attn_sb[:, ct, sc * P : (sc + 1) * P],
                                woT_sb[:, ct, en * 512 : (en + 1) * 512],
                                start=(ct == 0),
                                stop=(ct == 1),
                            )
                        o_sb = sb_out.tile([P, 512], F32, tag="osb")
                        nc.vector.tensor_copy(out=o_sb[:], in_=ps_o[:])
                        nc.sync.dma_start(
                            out_d[sc * P : (sc + 1) * P, en * 512 : (en + 1) * 512],
                            o_sb[:],
                        )

            for qc in range(NQ):
                # --- qk projection chunk nn = qc, ko-outer over two 2-bank
                # tiles (4 half-bank chains), so PE tracks DMA arrival ---
                pjA = pp_big.tile([P, 1024], F32, tag="big", name="pjA")
                pjB = pp_big.tile([P, 1024], F32, tag="big", name="pjB")
                for ko in range(KO):
                    for mm in range(4):
                        slot = pjA if mm < 2 else pjB
                        nc.tensor.matmul(
                            slot[:, (mm % 2) * 512 : (mm % 2 + 1) * 512],
                            wqkT_sb[:, ko, mm * P : (mm + 1) * P],
                            xT_sb[:, ko, qc * 512 : (qc + 1) * 512],
                            start=(ko == 0),
                            stop=(ko == KO - 1),
                            skip_group_check=True,
                        )
                nc.vector.tensor_copy(
                    out=qkT_sb[:, 0:2, qc * 512 : (qc + 1) * 512],
                    in_=pjA.rearrange("p (a b) -> p a b", a=2),
                )
                nc.vector.tensor_copy(
                    out=qkT_sb[:, 2:4, qc * 512 : (qc + 1) * 512],
                    in_=pjB.rearrange("p (a b) -> p a b", a=2),
                )

                # --- v projection for s-chunks 4qc..4qc+3 (4 bank chains) ---
                pvA = pp_big.tile([P, 1024], F32, tag="big", name="pvA")
                pvB = pp_big.tile([P, 1024], F32, tag="big", name="pvB")
                for ko in range(KO):
                    for j in range(4):
                        slot = pvA if j < 2 else pvB
                        sc = 4 * qc + j
                        nc.tensor.matmul(
                            slot[:, (j % 2) * 512 : (j % 2) * 512 + VC],
                            xT_sb[:, ko, sc * P : (sc + 1) * P],
                            wvT_sb[:, ko, :],
                            start=(ko == 0),
                            stop=(ko == KO - 1),
                            skip_group_check=True,
                        )
                for half, slot in ((0, pvA), (1, pvB)):
                    nc.vector.tensor_copy(
                        out=v_sb[:, 4 * qc + 2 * half : 4 * qc + 2 * half + 2, :, 0:DH],
                        in_=slot.rearrange("p (a h d) -> p a h d", a=2, h=8)[:, :, 0:HPC, :],
                    )

                # --- attention for q-chunk qc ---
                nkb = 4 * qc + 4  # causal: k blocks 0 .. 4qc+3
                for h in range(HPC):
                    hp = (h % 2) * DH  # partition base within the m-tile
                    mq = h // 2  # Q m-tile; K m-tile = 2 + h//2
                    avs = []
                    for kb0 in range(0, nkb, 2):
                        ps2 = pp_big.tile([P, 1024], F32, tag="big", name="ps2")
                        exp2 = sb_exp.tile([P, 1024], CDT, tag="exp")
                        offs = []
                        for half in (0, 1):
                            kb = kb0 + half
                            m = kb - 4 * qc  # >= 0 on diagonal straddlers
                            off = max(0, m) * P
                            offs.append(off)
                            nc.tensor.matmul(
                                ps2[:, half * 512 + off : half * 512 + 512],
                                qkT_sb[hp : hp + DH, 2 + mq, kb * P : (kb + 1) * P],
                                qkT_sb[hp : hp + DH, mq, qc * 512 + off : (qc + 1) * 512],
                                start=True,
                                stop=True,
                                skip_group_check=True,
                            )
                        if offs[0] == 0 and offs[1] == 0:
                            # dense pair: one 1024-wide exp
                            nc.scalar.activation(exp2[:], ps2[:], EXP, scale=0.125)
                        else:
                            for half, off in enumerate(offs):
                                lo = half * 512 + off
                                nc.scalar.activation(
                                    exp2[:, lo : half * 512 + 512],
                                    ps2[:, lo : half * 512 + 512],
                                    EXP,
                                    scale=0.125,
                                )
                        for half, off in enumerate(offs):
                            kb = kb0 + half
                            if kb - 4 * qc >= 0:
                                lo = half * 512 + off
                                nc.vector.tensor_mul(
                                    out=exp2[:, lo : lo + P],
                                    in0=exp2[:, lo : lo + P],
                                    in1=maskT_sb[:],
                                )
                            avs.append((exp2, half * 512 + off, off, kb))
                    ps_av = pp_av.tile([DH + 1, 512], F32, tag="av")
                    for j, (exp2, lo, off, kb) in enumerate(avs):
                        nc.tensor.matmul(
                            ps_av[:, off:512],
                            v_sb[:, kb, h, :],
                            exp2[:, lo : (lo - off) + 512],
                            start=(j == 0),
                            stop=(j == len(avs) - 1),
                            skip_group_check=True,
                        )
                    # normalize: out = av * (1/sums) broadcast over partitions
                    sums_sb = sb_small.tile([1, 512], F32, tag="sums")
                    nc.vector.tensor_copy(out=sums_sb[:], in_=ps_av[DH : DH + 1, :])
                    recip_f = sb_small.tile([1, 512], F32, tag="recipf")
                    nc.vector.reciprocal_approx_fast(out=recip_f[:], in_=sums_sb[:])
                    recip = sb_small.tile([1, 512], CDT, tag="recip")
                    nc.vector.tensor_copy(out=recip[:], in_=recip_f[:])
                    ps_b = pp_b.tile([DH, 512], F32, tag="b")
                    nc.tensor.matmul(ps_b[:], ones_sb[:], recip[:], start=True, stop=True)
                    bc_sb = sb_small.tile([DH, 512], F32, tag="bc")
                    nc.vector.tensor_copy(out=bc_sb[:], in_=ps_b[:])
                    nc.vector.tensor_mul(
                        out=attn_sb[hp : hp + DH, h // 2, qc * 512 : (qc + 1) * 512],
                        in0=ps_av[0:DH, :],
                        in1=bc_sb[:],
                    )

                # --- deferred output projection (previous q chunk) ---
                if qc > 0:
                    emit_outproj(qc - 1)
            emit_outproj(NQ - 1)

    nc.compile()
    return nc


def _get_nc():
    if "nc" not in _cache:
        _cache["nc"] = _build()
    return _cache["nc"]


def _shard(x, mask, Wqkv, Wo):
    cdt = _np_compute_dt()
    in_maps = []
    # binary mask for the transposed 128x128 diagonal block:
    # valid (mask==0) -> 1.0, masked (-inf/large-negative) -> 0.0
    maskT = np.ascontiguousarray((mask[0, 0, :P, :P].T >= 0).astype(cdt))
    for c in range(NCORES):
        b = c // 4
        g = c % 4
        heads = [4 * g + i for i in range(HPC)]
        q_rows = np.concatenate([np.arange(h * DH, (h + 1) * DH) for h in heads])
        k_rows = D + q_rows
        v_rows = 2 * D + q_rows
        qk_rows = np.concatenate([q_rows, k_rows])
        in_maps.append(
            {
                "xT": np.ascontiguousarray(x[b].T.astype(cdt)),
                "wqkT": np.ascontiguousarray(Wqkv[qk_rows, :].T.astype(cdt)),
                "wvT": np.ascontiguousarray(Wqkv[v_rows, :].T.astype(cdt)),
                "woT": np.ascontiguousarray(Wo[:, q_rows].T.astype(cdt)),
                "maskT": maskT,
            }
        )
    return in_maps


def kernel(x, mask, Wqkv, Wo, _trace=False):
    from concourse.bass_utils import run_bass_kernel_spmd

    x = np.asarray(x, dtype=np.float32)
    mask = np.asarray(mask, dtype=np.float32)
    Wqkv = np.asarray(Wqkv, dtype=np.float32)
    Wo = np.asarray(Wo, dtype=np.float32)

    nc = _get_nc()
    in_maps = _shard(x, mask, Wqkv, Wo)
    res = run_bass_kernel_spmd(nc, in_maps, core_ids=list(range(NCORES)), trace=_trace)
    _cache["last_result"] = res

    out = np.zeros((B, S, D), dtype=np.float32)
    for c in range(NCORES):
        out[c // 4] += res.results[c]["out"]
    return out


# revision 13
# speedup vs baseline: 1.3694x; 1.0685x over previous
"""Causal self-attention Trainium2 kernel (8-core SPMD).

Problem: x[2,2048,1024], causal mask, Wqkv[3072,1024], Wo[1024,1024], fp32.
  qkv = x @ Wqkv.T ; per-head causal softmax attention ; out = attn @ Wo.T

Sharding (data + tensor parallel, per the head dimension):
  core c -> batch b = c // 4, heads {4g..4g+3} with g = c % 4.
  Each core computes Q,K,V for its 4 heads (512 qk cols + 256 v cols of the
  projection), runs causal attention for those heads, and multiplies by the
  matching 256 columns of Wo, producing a partial [2048, 1024] output.
  Host sums the 4 partials per batch (the tensor-parallel reduction).

Kernel structure (per core):
  - bf16 matmul operands (PE 1 cyc/row), fp32 PSUM accumulation.
  - Projection chunks (ko-outer, so PE starts as soon as the first 128-row
    slices of x/w arrive) are interleaved with attention chunks: attention
    for q-chunk qc needs only projection chunks nn <= qc, so ACT exp work
    overlaps PE projection matmuls.
  - Scores are computed TRANSPOSED (scoresT[k, q], head pairs packed in the
    PE via partition-base row tiling) so AV needs no transposes. Score
    blocks go into 2-bank PSUM tiles (two k-blocks per tile) so one
    ACTIVATE exps 1024 columns, halving ACT instruction overhead.
  - Causality: strictly-upper blocks skipped; diagonal straddlers compute
    only the valid columns; the 128x128 diagonal sub-block is exp'd
    unmasked then multiplied by a binary mask tile (from the mask input).
  - V carries a ones column (65 cols/head): AV's partition 64 accumulates
    the softmax denominator for free. Normalization = fast-approx
    reciprocal (sums >= 1), broadcast over partitions via a K=1
    ones-matmul, one DVE multiply.
"""

import os

import numpy as np

S = 2048
D = 1024
DH = 64
B = 2
NCORES = 8
HPC = 4  # heads per core
QKC = 2 * HPC * DH  # 512 q+k projection columns per core
VC = HPC * DH  # 256 v columns per core
P = 128
KO = D // P  # 8 contraction tiles
NQ = S // 512  # 4 q-chunks of 512
NSC = S // P  # 16 s-chunks of 128

COMPUTE_DT = os.environ.get("ATTN_COMPUTE_DT", "bf16")  # bf16 | f32r

_cache = {}


def _np_compute_dt():
    if COMPUTE_DT == "bf16":
        import ml_dtypes

        return ml_dtypes.bfloat16
    return np.float32


def _build():
    import concourse.bacc as bacc
    import concourse.mybir as mybir
    import concourse.tile as tile

    F32 = mybir.dt.float32
    CDT = mybir.dt.bfloat16 if COMPUTE_DT == "bf16" else mybir.dt.float32r
    EXP = mybir.ActivationFunctionType.Exp

    nc = bacc.Bacc()
    xT_d = nc.dram_tensor("xT", [D, S], CDT, kind="ExternalInput")
    wqkT_d = nc.dram_tensor("wqkT", [D, QKC], CDT, kind="ExternalInput")
    wvT_d = nc.dram_tensor("wvT", [D, VC], CDT, kind="ExternalInput")
    woT_d = nc.dram_tensor("woT", [VC, D], CDT, kind="ExternalInput")
    maskT_d = nc.dram_tensor("maskT", [P, P], CDT, kind="ExternalInput")
    out_d = nc.dram_tensor("out", [S, D], F32, kind="ExternalOutput")

    with tile.TileContext(nc) as tc:
        with (
            tc.tile_pool(name="persist", bufs=1) as persist,
            tc.tile_pool(name="sb_small", bufs=3) as sb_small,
            tc.tile_pool(name="sb_exp", bufs=12) as sb_exp,
            tc.tile_pool(name="sb_out", bufs=3) as sb_out,
            tc.tile_pool(name="pp_big", bufs=2, space="PSUM") as pp_big,
            tc.tile_pool(name="pp_av", bufs=2, space="PSUM") as pp_av,
            tc.tile_pool(name="pp_o", bufs=2, space="PSUM") as pp_o,
        ):
            xT_sb = persist.tile([P, KO, S], CDT, tag="xT")
            wqkT_sb = persist.tile([P, KO, QKC], CDT, tag="wqkT")
            wvT_sb = persist.tile([P, KO, VC], CDT, tag="wvT")
            woT_sb = persist.tile([P, 2, D], CDT, tag="woT")
            maskT_sb = persist.tile([P, P], CDT, tag="maskT")
            qkT_sb = persist.tile([P, 4, S], CDT, tag="qkT")
            v_sb = persist.tile([P, NSC, HPC, DH + 1], CDT, tag="v")
            attn_sb = persist.tile([P, 2, S], CDT, tag="attn")

            # --- input DMAs: wqkT[ko]+xT[ko] pairs first (unblock proj ko
            # chains asap), across two queues; then wvT, mask, woT ---
            for ko in range(KO):
                e1, e2 = (nc.sync, nc.gpsimd) if ko % 2 == 0 else (nc.gpsimd, nc.sync)
                e1.dma_start(wqkT_sb[:, ko, :], wqkT_d[ko * P : (ko + 1) * P, :])
                e2.dma_start(xT_sb[:, ko, :], xT_d[ko * P : (ko + 1) * P, :])
                e1.dma_start(wvT_sb[:, ko, :], wvT_d[ko * P : (ko + 1) * P, :])
            nc.sync.dma_start(maskT_sb[:], maskT_d[:])
            nc.gpsimd.dma_start(woT_sb[:], woT_d.rearrange("(ct p) e -> p ct e", p=P))
            ones_f32 = persist.tile([P, DH], F32, tag="ones_f32")
            nc.vector.memset(ones_f32[:], 1.0)
            nc.vector.tensor_copy(
                out=v_sb[:, :, :, DH],
                in_=ones_f32[:, 0 : NSC * HPC].rearrange("p (a b) -> p a b", a=NSC),
            )

            def emit_outproj(qc):
                for si in range(4):
                    sc = qc * 4 + si
                    for en in range(2):
                        ps_o = pp_o.tile([P, 512], F32, tag="o")
                        for ct in range(2):
                            nc.tensor.matmul(
                                ps_o[:],
                                attn_sb[:, ct, sc * P : (sc + 1) * P],
                                woT_sb[:, ct, en * 512 : (en + 1) * 512],
                                start=(ct == 0),
                                stop=(ct == 1),
                            )
                        o_sb = sb_out.tile([P, 512], F32, tag="osb")
                        nc.vector.tensor_copy(out=o_sb[:], in_=ps_o[:])
                        nc.sync.dma_start(
                            out_d[sc * P : (sc + 1) * P, en * 512 : (en + 1) * 512],
                            o_sb[:],
                        )

            for qc in range(NQ):
                # --- qk projection chunk nn = qc, ko-outer over two 2-bank
                # tiles (4 half-bank chains), so PE tracks DMA arrival ---
                pjA = pp_big.tile([P, 1024], F32, tag="big", name="pjA")
                pjB = pp_big.tile([P, 1024], F32, tag="big", name="pjB")
                for ko in range(KO):
                    for mm in range(4):
                        slot = pjA if mm < 2 else pjB
                        nc.tensor.matmul(
                            slot[:, (mm % 2) * 512 : (mm % 2 + 1) * 512],
                            wqkT_sb[:, ko, mm * P : (mm + 1) * P],
                            xT_sb[:, ko, qc * 512 : (qc + 1) * 512],
                            start=(ko == 0),
                            stop=(ko == KO - 1),
                            skip_group_check=True,
                        )
                nc.vector.tensor_copy(
                    out=qkT_sb[:, 0:2, qc * 512 : (qc + 1) * 512],
                    in_=pjA.rearrange("p (a b) -> p a b", a=2),
                )
                nc.vector.tensor_copy(
                    out=qkT_sb[:, 2:4, qc * 512 : (qc + 1) * 512],
                    in_=pjB.rearrange("p (a b) -> p a b", a=2),
                )

                # --- v projection for s-chunks 4qc..4qc+3 (4 bank chains) ---
                pvA = pp_big.tile([P, 1024], F32, tag="big", name="pvA")
                pvB = pp_big.tile([P, 1024], F32, tag="big", name="pvB")
                for ko in range(KO):
                    for j in range(4):
                        slot = pvA if j < 2 else pvB
                        sc = 4 * qc + j
                        nc.tensor.matmul(
                            slot[:, (j % 2) * 512 : (j % 2) * 512 + VC],
                            xT_sb[:, ko, sc * P : (sc + 1) * P],
                            wvT_sb[:, ko, :],
                            start=(ko == 0),
                            stop=(ko == KO - 1),
                            skip_group_check=True,
                        )
                for half, slot in ((0, pvA), (1, pvB)):
                    nc.vector.tensor_copy(
                        out=v_sb[:, 4 * qc + 2 * half : 4 * qc + 2 * half + 2, :, 0:DH],
                        in_=slot.rearrange("p (a h d) -> p a h d", a=2, h=8)[:, :, 0:HPC, :],
                    )

                # --- attention for q-chunk qc ---
                nkb = 4 * qc + 4  # causal: k blocks 0 .. 4qc+3
                for h in range(HPC):
                    hp = (h % 2) * DH  # partition base within the m-tile
                    mq = h // 2  # Q m-tile; K m-tile = 2 + h//2
                    avs = []
                    for kb0 in range(0, nkb, 2):
                        ps2 = pp_big.tile([P, 1024], F32, tag="big", name="ps2")
                        exp2 = sb_exp.tile([P, 1024], CDT, tag="exp")
                        offs = []
                        for half in (0, 1):
                            kb = kb0 + half
                            m = kb - 4 * qc  # >= 0 on diagonal straddlers
                            off = max(0, m) * P
                            offs.append(off)
                            nc.tensor.matmul(
                                ps2[:, half * 512 + off : half * 512 + 512],
                                qkT_sb[hp : hp + DH, 2 + mq, kb * P : (kb + 1) * P],
                                qkT_sb[hp : hp + DH, mq, qc * 512 + off : (qc + 1) * 512],
                                start=True,
                                stop=True,
                                skip_group_check=True,
                            )
                        if offs[0] == 0 and offs[1] == 0:
                            # dense pair: one 1024-wide exp
                            nc.scalar.activation(exp2[:], ps2[:], EXP, scale=0.125)
                        else:
                            for half, off in enumerate(offs):
                                lo = half * 512 + off
                                nc.scalar.activation(
                                    exp2[:, lo : half * 512 + 512],
                                    ps2[:, lo : half * 512 + 512],
                                    EXP,
                                    scale=0.125,
                                )
                        for half, off in enumerate(offs):
                            kb = kb0 + half
                            if kb - 4 * qc >= 0:
                                lo = half * 512 + off
                                nc.vector.tensor_mul(
                                    out=exp2[:, lo : lo + P],
                                    in0=exp2[:, lo : lo + P],
                                    in1=maskT_sb[:],
                                )
                            avs.append((exp2, half * 512 + off, off, kb))
                    ps_av = pp_av.tile([DH + 1, 512], F32, tag="av")
                    for j, (exp2, lo, off, kb) in enumerate(avs):
                        nc.tensor.matmul(
                            ps_av[:, off:512],
                            v_sb[:, kb, h, :],
                            exp2[:, lo : (lo - off) + 512],
                            start=(j == 0),
                            stop=(j == len(avs) - 1),
                            skip_group_check=True,
                        )
                    # normalize: out = av * (1/sums) broadcast over partitions
                    sums_sb = sb_small.tile([1, 512], F32, tag="sums")
                    nc.vector.tensor_copy(out=sums_sb[:], in_=ps_av[DH : DH + 1, :])
                    recip_f = sb_small.tile([1, 512], F32, tag="recipf")
                    nc.vector.reciprocal_approx_fast(out=recip_f[:], in_=sums_sb[:])
                    bc_sb = sb_small.tile([DH, 512], F32, tag="bc")
                    nc.gpsimd.partition_broadcast(bc_sb[:], recip_f[:])
                    nc.vector.tensor_mul(
                        out=attn_sb[hp : hp + DH, h // 2, qc * 512 : (qc + 1) * 512],
                        in0=ps_av[0:DH, :],
                        in1=bc_sb[:],
                    )

                # --- deferred output projection (previous q chunk) ---
                if qc > 0:
                    emit_outproj(qc - 1)
            emit_outproj(NQ - 1)

    nc.compile()
    return nc


def _get_nc():
    if "nc" not in _cache:
        _cache["nc"] = _build()
    return _cache["nc"]


def _shard(x, mask, Wqkv, Wo):
    cdt = _np_compute_dt()
    in_maps = []
    # binary mask for the transposed 128x128 diagonal block:
    # valid (mask==0) -> 1.0, masked (-inf/large-negative) -> 0.0
    maskT = np.ascontiguousarray((mask[0, 0, :P, :P].T >= 0).astype(cdt))
    for c in range(NCORES):
        b = c // 4
        g = c % 4
        heads = [4 * g + i for i in range(HPC)]
        q_rows = np.concatenate([np.arange(h * DH, (h + 1) * DH) for h in heads])
        k_rows = D + q_rows
        v_rows = 2 * D + q_rows
        qk_rows = np.concatenate([q_rows, k_rows])
        in_maps.append(
            {
                "xT": np.ascontiguousarray(x[b].T.astype(cdt)),
                "wqkT": np.ascontiguousarray(Wqkv[qk_rows, :].T.astype(cdt)),
                "wvT": np.ascontiguousarray(Wqkv[v_rows, :].T.astype(cdt)),
                "woT": np.ascontiguousarray(Wo[:, q_rows].T.astype(cdt)),
                "maskT": maskT,
            }
        )
    return in_maps


def kernel(x, mask, Wqkv, Wo, _trace=False):
    from concourse.bass_utils import run_bass_kernel_spmd

    x = np.asarray(x, dtype=np.float32)
    mask = np.asarray(mask, dtype=np.float32)
    Wqkv = np.asarray(Wqkv, dtype=np.float32)
    Wo = np.asarray(Wo, dtype=np.float32)

    nc = _get_nc()
    in_maps = _shard(x, mask, Wqkv, Wo)
    res = run_bass_kernel_spmd(nc, in_maps, core_ids=list(range(NCORES)), trace=_trace)
    _cache["last_result"] = res

    out = np.zeros((B, S, D), dtype=np.float32)
    for c in range(NCORES):
        out[c // 4] += res.results[c]["out"]
    return out
